# revision 38
# baseline (speedup 1.0000x reference)
"""BandSplit layer Trainium2 kernel.

Computes, for input [16, 1000, 257]:
  - 28 frequency bands: 8 bands x 4 bins (bins 0..31), 12 x 8 (32..127),
    8 x 16 (128..255)  (bin 256 unused)
  - per-band layernorm over the band's bins (eps=1e-3), with per-band
    gamma/beta, then a per-band dense projection [c] -> [128] plus bias.
  - output [16, 1000, 28, 128]

Strategy: data-parallel over batch across 8 NeuronCores (2 batches =
2000 tokens per core).  gamma is folded into the dense weights and
beta/bias into a single per-output bias on the host, so the device does
plain layernorm + matmul.  Per 128-token tile:
  load token-major [128, 257] -> LN stats via free-dim reduces (DVE) ->
  normalize in place (GpSimd) -> PE transpose to [bins, tok] ->
  7 block-diagonal fp32r matmuls (N=512) against a packed [128, 3584]
  weight matrix, two chunks per 2-bank PSUM tile -> PSUM->SBUF drains
  split across Scalar/Vector engines -> quarter-tile DMAs out so the
  DMA engines stay fed continuously.
"""

import sys

import numpy as np

for _p in ("/opt/trn_rl_repo", "/root/.axon_site/_ro/trn_rl_repo"):
    if _p not in sys.path:
        sys.path.append(_p)

EPS = 1e-3
D = 128
GROUPS = [(8, 4, 0), (12, 8, 32), (8, 16, 128)]  # (n_bands, bins_per_band, start_bin)
B, T, F = 16, 1000, 257
N_CORES = 8
TOK = B * T // N_CORES  # tokens per core = 2000
NB = sum(n for n, _, _ in GROUPS)  # 28 bands
OUT_COLS = NB * D  # 3584
P = 128
N_CHUNK = 512  # matmul free-dim chunk (one PSUM bank)
N_CHUNKS = OUT_COLS // N_CHUNK  # 7
# Per-band layout: (ktile, krow0, c) per band; ktile 0 = bins 0..127,
# ktile 1 = bins 128..255. Output cols for band i are [i*128, (i+1)*128).
_BANDS = []
for _n, _c, _s in GROUPS:
    for _k in range(_n):
        _bin0 = _s + _k * _c
        _BANDS.append((_bin0 // 128, _bin0 % 128, _c))

_STATE = {}


def _build(has_bias):
    """Trace + compile the Bass kernel (cached per process)."""
    from contextlib import ExitStack

    import concourse.bass as bass
    import concourse.tile as tile
    from concourse import bacc, mybir

    f32 = mybir.dt.float32
    f32r_dram = mybir.dt.float32r
    nc = bacc.Bacc(
        "TRN2", target_bir_lowering=False, debug=False, num_devices=N_CORES
    )
    x_d = nc.dram_tensor("x", [TOK, F], f32, kind="ExternalInput").ap()
    # Declared float32r (same 4-byte layout): DMA straight to the fp32r
    # weight tile with no on-chip rounding pass on the startup critical path.
    w_d = nc.dram_tensor("wpack", [P, OUT_COLS], f32r_dram, kind="ExternalInput").ap()
    id_d = nc.dram_tensor("ident", [P, P], f32, kind="ExternalInput").ap()
    ci_d = nc.dram_tensor("cinv2", [1, 2 * NB], f32, kind="ExternalInput").ap()
    if has_bias:
        b_d = nc.dram_tensor("bias", [1, OUT_COLS], f32, kind="ExternalInput").ap()
    out_d = nc.dram_tensor("out", [TOK, OUT_COLS], f32, kind="ExternalOutput").ap()

    n_tiles = (TOK + P - 1) // P

    with tile.TileContext(nc) as tc, ExitStack() as ctx:
        const = ctx.enter_context(tc.tile_pool(name="const", bufs=1))
        xin = ctx.enter_context(tc.tile_pool(name="xin", bufs=7))
        sqp = ctx.enter_context(tc.tile_pool(name="sqp", bufs=3))
        ln = ctx.enter_context(tc.tile_pool(name="ln", bufs=3))
        xnt = ctx.enter_context(tc.tile_pool(name="xnt", bufs=3))
        outp = ctx.enter_context(tc.tile_pool(name="outp", bufs=6))
        ps_tr = ctx.enter_context(tc.tile_pool(name="ps_tr", bufs=2, space="PSUM"))
        ps_mm = ctx.enter_context(tc.tile_pool(name="ps_mm", bufs=3, space="PSUM"))

        f32r = mybir.dt.float32r

        # Prefetch the first input tiles on the sync queue ahead of the
        # weight pieces: the x(0) load heads the startup critical path.
        xts = {}

        def load_x(it, engine=None):
            if it in xts or it >= n_tiles:
                return
            t0 = it * P
            tn = min(P, TOK - t0)
            xt = xin.tile([P, F], f32, tag="xt")
            (engine or nc.gpsimd).dma_start(out=xt[:tn, :], in_=x_d[t0 : t0 + tn, :])
            xts[it] = xt

        for it in range(3):
            load_x(it, engine=nc.sync)

        w_sbr = const.tile([P, OUT_COLS], f32r)
        nc.sync.dma_start(out=w_sbr[:, 0:1024], in_=w_d[:, 0:1024])

        ident = const.tile([P, P], f32)
        nc.sync.dma_start(out=ident[:], in_=id_d)
        eps_t = const.tile([P, 1], f32)
        nc.vector.memset(eps_t[:], EPS)
        # 1/c per band (twice: for sums and sumsq), replicated across
        # partitions: mean|ex2 = (sums|sumsq) * cinv2 in one DVE op.
        cinv2 = const.tile([P, 2 * NB], f32)
        nc.sync.dma_start(
            out=cinv2[:],
            in_=bass.AP(tensor=ci_d.tensor, offset=ci_d.offset, ap=[[0, P], ci_d.ap[1]]),
        )
        # Tiles 3-5 also prefetch on sync: the GpSimd queue then opens with
        # tile 0's square instead of a run of load-issues.
        for it in range(3, 6):
            load_x(it, engine=nc.sync)
        for piece in range(1, 4):
            c0 = piece * 1024
            c1 = min(OUT_COLS, c0 + 1024)
            nc.sync.dma_start(out=w_sbr[:, c0:c1], in_=w_d[:, c0:c1])
        if has_bias:
            bias_sb = const.tile([P, OUT_COLS], f32)
            nc.sync.dma_start(
                out=bias_sb[:],
                in_=bass.AP(
                    tensor=b_d.tensor, offset=b_d.offset, ap=[[0, P], b_d.ap[1]]
                ),
            )

        for it in range(n_tiles):
            t0 = it * P
            tn = min(P, TOK - t0)

            load_x(it)
            xt = xts.pop(it)

            # --- layernorm statistics (per token x band) ---
            sq = sqp.tile([P, 256], f32)
            nc.gpsimd.tensor_mul(sq[:tn, :], xt[:tn, 0:256], xt[:tn, 0:256])

            if it == 0:
                # PE warm-up: dummy transposes of the first square tile start
                # the HAM activity clock ~3us before the first real matmuls,
                # so those run at the warm 2.4GHz clock instead of 1.2GHz.
                for h in range(2):
                    ptw = ps_tr.tile([P, P], f32, tag="pt")
                    nc.tensor.transpose(
                        ptw[:, :tn], sq[:tn, h * P : (h + 1) * P], ident[:tn, :tn]
                    )

            ss = ln.tile([P, 2, NB], f32)
            b0 = 0
            for n, c, s in GROUPS:
                xg = xt[:tn, s : s + n * c].rearrange("p (g c) -> p g c", g=n)
                sg = sq[:tn, s : s + n * c].rearrange("p (g c) -> p g c", g=n)
                nc.vector.reduce_sum(
                    out=ss[:tn, 0, b0 : b0 + n], in_=xg, axis=mybir.AxisListType.X
                )
                nc.vector.reduce_sum(
                    out=ss[:tn, 1, b0 : b0 + n], in_=sg, axis=mybir.AxisListType.X
                )
                b0 += n

            me = ln.tile([P, 2, NB], f32)  # me[:,0]=mean, me[:,1]=E[x^2]
            nc.vector.tensor_mul(
                me[:tn].rearrange("p a b -> p (a b)"), ss[:tn].rearrange("p a b -> p (a b)"), cinv2[:tn]
            )
            mean = me[:, 0]
            var = ln.tile([P, NB], f32)
            nc.vector.tensor_mul(var[:tn, :], mean[:tn, :], mean[:tn, :])
            nc.vector.tensor_sub(var[:tn, :], me[:tn, 1, :], var[:tn, :])
            rstd = ln.tile([P, NB], f32)
            nc.scalar.activation(
                out=rstd[:tn, :],
                in_=var[:tn, :],
                func=mybir.ActivationFunctionType.Sqrt,
                bias=eps_t[:tn, :],
                scale=1.0,
            )
            nc.vector.reciprocal(out=rstd[:tn, :], in_=rstd[:tn, :])

            # --- normalize in place: xn = (x - mean) * rstd (GpSimd: frees DVE) ---
            b0 = 0
            for n, c, s in GROUPS:
                xg = xt[:tn, s : s + n * c].rearrange("p (g c) -> p g c", g=n)
                nc.gpsimd.tensor_sub(
                    xg, xg, mean[:tn, b0 : b0 + n].to_broadcast((tn, n, c))
                )
                nc.gpsimd.tensor_mul(
                    xg, xg, rstd[:tn, b0 : b0 + n].to_broadcast((tn, n, c))
                )
                b0 += n

            load_x(it + 3)  # prefetch ahead (xin bufs cover the lookahead)

            # --- transpose to [bins, tok] (two 128-col halves) ---
            xnt_h = []
            for h in range(2):
                pt = ps_tr.tile([P, P], f32, tag="pt")
                nc.tensor.transpose(
                    pt[:, :tn], xt[:tn, h * P : (h + 1) * P], ident[:tn, :tn]
                )
                st = xnt.tile([P, P], f32r, tag=f"xnt{h}")
                nc.scalar.copy(st[:, :tn], pt[:, :tn])
                xnt_h.append(st)

            # --- 7 fp32r matmuls in 2-bank PSUM pairs + drains + quarter DMAs ---
            ot = outp.tile([P, OUT_COLS], f32)
            for pair in range(4):
                js = [j for j in (2 * pair, 2 * pair + 1) if j < N_CHUNKS]
                pm = ps_mm.tile([P, 2 * N_CHUNK], f32, tag="pm")
                for k, j in enumerate(js):
                    lhsT = xnt_h[0] if j * N_CHUNK < 2560 else xnt_h[1]
                    wcol = j * N_CHUNK
                    nc.tensor.matmul(
                        pm[:tn, k * N_CHUNK : (k + 1) * N_CHUNK],
                        lhsT[:, :tn],
                        w_sbr[:, wcol : wcol + N_CHUNK],
                        start=True,
                        stop=True,
                    )
                c0 = 2 * pair * N_CHUNK
                c1 = c0 + len(js) * N_CHUNK
                osl = ot[:tn, c0:c1]
                pms = pm[:tn, 0 : (c1 - c0)]
                if has_bias:
                    nc.vector.tensor_add(osl, pms, bias_sb[:tn, c0:c1])
                elif (pair + it) % 2 == 0:
                    nc.scalar.copy(osl, pms)
                else:
                    nc.vector.tensor_copy(osl, pms)
                # Ship this quarter as soon as its drain lands: keeps the DMA
                # engines fed through the whole compute window.
                nc.sync.dma_start(out=out_d[t0 : t0 + tn, c0:c1], in_=ot[:tn, c0:c1])

    nc.compile()
    return nc


def _get_nc(has_bias):
    key = ("nc", has_bias)
    if key not in _STATE:
        _STATE[key] = _build(has_bias)
    return _STATE[key]


def _pack_weights(inputs):
    """Fold gamma into W, beta/b into bias; pack block-diagonal [128, 3584]."""
    wpack = np.zeros((P, OUT_COLS), dtype=np.float32)
    bias = np.zeros((OUT_COLS,), dtype=np.float32)
    bi = 0
    for gi, (n, c, _s) in enumerate(GROUPS, start=1):
        gamma = np.asarray(inputs[f"gamma{gi}"], dtype=np.float32)  # [n, c]
        beta = np.asarray(inputs[f"beta{gi}"], dtype=np.float32)  # [n, c]
        W = np.asarray(inputs[f"W{gi}"], dtype=np.float32)  # [n, c, D]
        b = np.asarray(inputs[f"b{gi}"], dtype=np.float32)  # [n, D]
        for k in range(n):
            _ktile, krow0, cc = _BANDS[bi]
            assert cc == c
            c0, c1 = bi * D, (bi + 1) * D
            wpack[krow0 : krow0 + c, c0:c1] = gamma[k][:, None] * W[k]
            bias[c0:c1] = beta[k] @ W[k] + b[k]
            bi += 1
    return wpack, bias


def _cinv2():
    ci = np.zeros((1, 2 * NB), dtype=np.float32)
    for half in range(2):
        b0 = 0
        for n, c, _s in GROUPS:
            ci[0, half * NB + b0 : half * NB + b0 + n] = 1.0 / c
            b0 += n
    return ci


def _prepare(inputs):
    """-> (nc, in_maps) for the 8 cores."""
    x = np.asarray(inputs["inputs"], dtype=np.float32)
    assert x.shape == (B, T, F), x.shape
    wpack, bias = _pack_weights(inputs)
    has_bias = bool(np.any(bias != 0.0))

    nc = _get_nc(has_bias)

    xflat = np.ascontiguousarray(x.reshape(B * T, F))
    ident = np.eye(P, dtype=np.float32)
    cinv2 = _cinv2()
    in_maps = []
    for c in range(N_CORES):
        m = {
            "x": xflat[c * TOK : (c + 1) * TOK],
            "wpack": wpack,
            "ident": ident,
            "cinv2": cinv2,
        }
        if has_bias:
            m["bias"] = bias.reshape(1, OUT_COLS)
        in_maps.append(m)
    return nc, in_maps


def kernel(**inputs):
    from concourse.bass_utils import run_bass_kernel_spmd

    nc, in_maps = _prepare(inputs)
    res = run_bass_kernel_spmd(nc, in_maps, list(range(N_CORES))).results
    out = np.concatenate([r["out"] for r in res], axis=0)
    return out.reshape(B, T, NB, D)
